# revision 13
# baseline (speedup 1.0000x reference)
"""Bass/Trainium2 kernel for nn_BasicSoftmaxRouter (noisy top-k MoE router).

Computes, for x:[4,4096,2048] f32, w_g/w_noise:[8,2048] f32, eps:[4,4096,8] f32:
    logits = x @ w_g.T + softplus(x @ w_noise.T) * eps
    return top_k(logits, k=2)  ->  (values [4,4096,2] f32, indices [4,4096,2] int32)

Strategy: data-parallel over 8 NeuronCores; 2048 tokens per core. Host
pre-transposes each x shard to [D, T] so the contraction dim lands on SBUF
partitions and every DMA is fully contiguous.

Matmul precision/speed: fp32 on the PE costs 4 cycles/row (2 half-speed
passes). Instead we use a scaled fp16 hi/lo split at 3 passes x 1 cycle/row:
    x_s = 16*x = xh + xl   (fp16 hi + residual lo, ~22 mantissa bits)
    w_s = 64*w = wh + wl
    x_s @ w_s ~= xh@wh + xl@wh + xh@wl     (xl@wl ~ 2^-24, dropped)
The power-of-two pre-scales keep every residual in fp16 normal range (w ~
1/sqrt(2048) would otherwise make wl subnormal) and are undone for free via
the ACT scale parameter / a fused scalar_tensor_tensor multiply (1/1024).
Logit error ~1e-6 -- same grade as the fp32 reference itself.

On-device per core:
  - matmul: lhsT = w chunk [128, 16] fp16 (stationary), rhs = x [128, 512]
    fp16 (moving), 3 passes x 16 K-chunks accumulating into PSUM [16, 512]
    per 512-token group.
  - x DMAs are split by token-range so early groups' postprocessing overlaps
    the later groups' loads (shrinks the serial tail).
  - postprocess: PSUM->SBUF copy, PE transpose to [128 tokens, 16],
    softplus = Ln(Exp(z/1024)+1) on ACT, noise mult + descaled add on DVE,
    then HW max8/max_index for the top-2 values + indices.
"""

import os

import numpy as np

import concourse.bacc as bacc
import concourse.mybir as mybir
import concourse.tile as tile
from concourse.bass_utils import run_bass_kernel_spmd
from concourse.masks import make_identity

N_CORES = 8
B, S, D, E = 4, 4096, 2048, 8
TOKENS = B * S          # 16384
T = TOKENS // N_CORES   # 2048 tokens per core
M = 2 * E               # 16 stacked outputs: w_g logits ++ w_noise logits
P = 128
N_CHUNKS = D // P       # 16 contraction chunks
GROUP = 512             # tokens per PSUM accumulation group
N_GROUPS = T // GROUP   # 4
TPG = GROUP // P        # 4 token-tiles (of 128) per group
N_TILES = T // P        # 16
TOPK = 2

F32 = mybir.dt.float32
F16 = mybir.dt.float16

X_SCALE = 16.0          # x pre-scale (power of 2)
W_SCALE = 64.0          # w pre-scale (power of 2)
DESCALE = 1.0 / (X_SCALE * W_SCALE)   # 2^-10

# "f16x3" (scaled fp16 hi/lo, 3 passes) or "f32" (native, 4 cyc/row)
MM_MODE = os.environ.get("ROUTER_MM_MODE", "f16x3")
# x DMA split: groups per DMA segment (4 = one DMA per chunk, 2 = halves,
# 1 = quarters). Finer splits let early-group postprocess overlap later loads.
SPLIT = int(os.environ.get("ROUTER_SPLIT", "2"))

_cache: dict = {}

# test.py reads this for profiling info after calling kernel()
last_results = None


def _build(reps: int = 1, mm_mode: str | None = None, split: int | None = None,
           xbufs: int | None = None):
    mode = mm_mode or MM_MODE
    f16 = mode == "f16x3"
    nc = bacc.Bacc(None, target_bir_lowering=False)

    if f16:
        # xp[:, 0, :] = hi half, xp[:, 1, :] = lo residual (both fp16, scaled)
        xp_d = nc.dram_tensor("xp", [D, 2, T], F16, kind="ExternalInput")
        wh_d = nc.dram_tensor("wh", [P, N_CHUNKS, M], F16, kind="ExternalInput")
        wl_d = nc.dram_tensor("wl", [P, N_CHUNKS, M], F16, kind="ExternalInput")
    else:
        xt = nc.dram_tensor("xt", [D, T], F32, kind="ExternalInput")
        wi = nc.dram_tensor("wi", [P, N_CHUNKS, M], F32, kind="ExternalInput")
    epsi = nc.dram_tensor("epsi", [P, N_TILES, E], F32, kind="ExternalInput")
    out_o = nc.dram_tensor("out_o", [P, N_TILES, 2 * TOPK], F32,
                           kind="ExternalOutput")

    descale = DESCALE if f16 else 1.0
    gseg = split or SPLIT          # groups per DMA segment
    n_seg = N_GROUPS // gseg       # DMA segments per chunk
    seg_tok = gseg * GROUP         # tokens per segment

    with tile.TileContext(nc) as tc:
        with (
            tc.tile_pool(name="const", bufs=1) as cpool,
            tc.tile_pool(name="xbuf", bufs=xbufs or (2 * n_seg + 2)) as xpool,
            tc.tile_pool(name="work", bufs=3) as wpool,
            tc.tile_pool(name="outb", bufs=2) as opool,
            tc.tile_pool(name="mm", bufs=N_GROUPS, space="PSUM") as mmpool,
            tc.tile_pool(name="tp", bufs=2, space="PSUM") as tppool,
        ):
            if f16:
                wh_sb = cpool.tile([P, N_CHUNKS, M], F16)
                nc.sync.dma_start(wh_sb[:], wh_d[:])
                wl_sb = cpool.tile([P, N_CHUNKS, M], F16)
                nc.sync.dma_start(wl_sb[:], wl_d[:])
            else:
                w_sb = cpool.tile([P, N_CHUNKS, M], F32)
                nc.sync.dma_start(w_sb[:], wi[:])
            eps_sb = cpool.tile([P, N_TILES, E], F32)
            nc.sync.dma_start(eps_sb[:], epsi[:])
            ident = cpool.tile([M, M], F32)
            make_identity(nc, ident)
            # preload the exp/ln ACT table set off the critical path
            warm = cpool.tile([1, 1], F32)
            nc.vector.memset(warm[:], 0.0)
            nc.scalar.activation(warm[:], warm[:],
                                 mybir.ActivationFunctionType.Exp)

            for _ in range(reps):
                vals_w = opool.tile([P, N_TILES, 8], F32, tag="vw", name="vals_w")
                idx_w = opool.tile([P, N_TILES, 8], mybir.dt.uint32, tag="iw",
                                   name="idx_w")

                psums = [
                    mmpool.tile([M, GROUP], F32, name=f"ps{q}", tag="ps")
                    for q in range(N_GROUPS)
                ]

                def do_group(q):
                    lg = wpool.tile([M, GROUP], F32, tag="lg", name=f"lg{q}")
                    nc.vector.tensor_copy(lg[:], psums[q][:])

                    pt = tppool.tile([P, TPG * M], F32, tag="pt", name=f"pt{q}")
                    for t in range(TPG):
                        nc.tensor.transpose(
                            pt[:, t * M:(t + 1) * M], lg[:, t * P:(t + 1) * P],
                            ident,
                        )
                    ptv = pt.rearrange("p (t m) -> p t m", m=M)

                    # softplus(z) = ln(1 + exp(z)); no Softplus ACT table in
                    # bass, but Exp and Ln share natural_log_exp_and_others.
                    # The matmul pre-scale is undone by Exp's free scale.
                    ex = wpool.tile([P, TPG, E], F32, tag="ex", name=f"ex{q}")
                    nc.scalar.activation(
                        ex[:], ptv[:, :, E:M], mybir.ActivationFunctionType.Exp,
                        scale=descale,
                    )
                    u = wpool.tile([P, TPG, E], F32, tag="u", name=f"u{q}")
                    nc.scalar.activation(
                        u[:], ex[:], mybir.ActivationFunctionType.Ln, bias=1.0
                    )
                    nz = wpool.tile([P, TPG, E], F32, tag="nz", name=f"nz{q}")
                    nc.vector.tensor_tensor(
                        nz[:], u[:], eps_sb[:, q * TPG:(q + 1) * TPG, :],
                        mybir.AluOpType.mult,
                    )
                    L = wpool.tile([P, TPG, E], F32, tag="L", name=f"L{q}")
                    nc.vector.scalar_tensor_tensor(
                        L[:], ptv[:, :, 0:E], descale, nz[:],
                        mybir.AluOpType.mult, mybir.AluOpType.add,
                    )

                    po = opool.tile([P, TPG, 2 * TOPK], F32, tag="po",
                                    name=f"po{q}")
                    for t in range(TPG):
                        g = q * TPG + t
                        nc.vector.max(vals_w[:, g, :], L[:, t, :])
                        nc.vector.max_index(
                            idx_w[:, g, :], vals_w[:, g, :], L[:, t, :]
                        )
                        nc.vector.tensor_copy(
                            po[:, t, 0:TOPK], vals_w[:, g, 0:TOPK]
                        )
                        nc.vector.tensor_copy(
                            po[:, t, TOPK:2 * TOPK],
                            idx_w.bitcast(F32)[:, g, 0:TOPK],
                        )
                    nc.sync.dma_start(
                        out_o[:, q * TPG:(q + 1) * TPG, :], po[:]
                    )

                for s in range(n_seg):
                    for c in range(N_CHUNKS):
                        tok = slice(s * seg_tok, (s + 1) * seg_tok)
                        row = slice(c * P, (c + 1) * P)
                        if f16:
                            xp_sb = xpool.tile([P, 2, seg_tok], F16, tag="xh",
                                               name=f"xp{s}_{c}")
                            nc.sync.dma_start(xp_sb[:], xp_d[row, :, tok])
                            xh_sb = xp_sb[:, 0, :]
                            xl_sb = xp_sb[:, 1, :]
                            passes = [
                                (wh_sb[:, c, :], xh_sb),
                                (wh_sb[:, c, :], xl_sb),
                                (wl_sb[:, c, :], xh_sb),
                            ]
                        else:
                            x_sb = xpool.tile([P, seg_tok], F32, tag="xh",
                                              name=f"x{s}_{c}")
                            nc.sync.dma_start(x_sb[:], xt[row, tok])
                            passes = [(w_sb[:, c, :], x_sb)]
                        np_ = len(passes)
                        for qq in range(gseg):
                            q = s * gseg + qq
                            for i, (lhsT, xsb) in enumerate(passes):
                                nc.tensor.matmul(
                                    psums[q][:],
                                    lhsT=lhsT,
                                    rhs=xsb[:, qq * GROUP:(qq + 1) * GROUP],
                                    start=(c == 0 and i == 0),
                                    stop=(c == N_CHUNKS - 1 and i == np_ - 1),
                                )
                    for qq in range(gseg):
                        do_group(s * gseg + qq)
    nc.compile()
    return nc


def _get_nc():
    if "nc" not in _cache:
        _cache["nc"] = _build()
    return _cache["nc"]


def _split_f16(a: np.ndarray, scale: float) -> tuple[np.ndarray, np.ndarray]:
    s = (a * scale).astype(np.float32)
    hi = s.astype(np.float16)
    lo = (s - hi.astype(np.float32)).astype(np.float16)
    return hi, lo


def kernel(**inputs) -> tuple[np.ndarray, np.ndarray]:
    global last_results
    x = np.ascontiguousarray(np.asarray(inputs["x"], dtype=np.float32))
    w_g = np.asarray(inputs["w_g"], dtype=np.float32)
    w_noise = np.asarray(inputs["w_noise"], dtype=np.float32)
    eps = np.ascontiguousarray(np.asarray(inputs["eps"], dtype=np.float32))

    xf = x.reshape(TOKENS, D)
    ef = eps.reshape(TOKENS, E)
    w_cat = np.concatenate([w_g, w_noise], axis=0)  # [M, D]
    # wi[p, c, m] == w_cat[m, c*128 + p]
    wi = np.ascontiguousarray(w_cat.T.reshape(N_CHUNKS, P, M).transpose(1, 0, 2))

    f16 = MM_MODE == "f16x3"
    if f16:
        wh, wl = _split_f16(wi, W_SCALE)

    in_maps = []
    for i in range(N_CORES):
        xs = xf[i * T:(i + 1) * T]                     # [T, D]
        xti = np.ascontiguousarray(xs.T)               # [D, T]
        es = np.ascontiguousarray(
            ef[i * T:(i + 1) * T].reshape(N_TILES, P, E).transpose(1, 0, 2)
        )                                              # [P, N_TILES, E]
        if f16:
            xhi, xlo = _split_f16(xti, X_SCALE)
            xp = np.ascontiguousarray(np.stack([xhi, xlo], axis=1))  # [D,2,T]
            in_maps.append({"xp": xp, "wh": wh, "wl": wl, "epsi": es})
        else:
            in_maps.append({"xt": xti, "wi": wi, "epsi": es})

    nc = _get_nc()
    res = run_bass_kernel_spmd(
        nc,
        in_maps,
        core_ids=list(range(N_CORES)),
        trace=bool(int(os.environ.get("ROUTER_TRACE", "0"))),
    )
    last_results = res

    vals = np.empty((TOKENS, TOPK), np.float32)
    idx = np.empty((TOKENS, TOPK), np.int32)
    for i, r in enumerate(res.results):
        po = r["out_o"]                                 # [P, N_TILES, 4]
        vals[i * T:(i + 1) * T] = (
            po[:, :, 0:TOPK].transpose(1, 0, 2).reshape(T, TOPK)
        )
        idx[i * T:(i + 1) * T] = (
            po[:, :, TOPK:2 * TOPK].view(np.int32)
            .transpose(1, 0, 2).reshape(T, TOPK)
        )
    return vals.reshape(B, S, TOPK), idx.reshape(B, S, TOPK)
